# revision 11
# baseline (speedup 1.0000x reference)
"""DeepSeekV3 router (moe_routing) Bass kernel for 8x TRN2 NeuronCores.

Data-parallel over tokens (T sharded 8 ways); kernel_DE/bias_E replicated.

z = x@W via one fp32r main pass plus one fp8 DoubleRow pass:
  main: xr_s * W_r           (fp32r, 12-bit significand, scaled 2^10)
  corr: xe8 * W_r8 + x8 * W_e8  (fp8e4m3 DoubleRow, both terms in one pass)
with xe = x - fp32r(x), We = W - fp32r(W). Residual error ~2^-17 relative,
at 256+128 matmul cycles per 128x128 chunk instead of fp32's 1024.
"""

import numpy as np

import concourse.bass as bass
import concourse.mybir as mybir
from concourse import bacc
from concourse.bass_utils import run_bass_kernel_spmd
from concourse.masks import make_identity
from concourse.tile import TileContext

F32 = mybir.dt.float32
F32R = mybir.dt.float32r
F8 = mybir.dt.float8e4
I32 = mybir.dt.int32
U32 = mybir.dt.uint32

T, D, E = 16384, 7168, 256
N_CORES = 8
TOP_K = 8
N_GROUPS = 8
TOPK_GROUPS = 4
EPG = E // N_GROUPS
SCALE = 2.5

P = 128
TS = T // N_CORES
KC = D // P                # 56 contraction chunks
TG = 8                     # chunks per group (PSUM stage + xt granularity)
NG = KC // TG              # 7 groups per tile
MM_LAG = 2                 # matmul groups lag transposes by this many steps

SCALE_X = 2.0**10          # x upscale inside xr_s (undone after matmul)
SH_W8 = 2.0**7             # W_r8 fp8 scale
SH_WE8 = 2.0**17           # W_e8 fp8 scale
# corr PSUM carries 2^17 * (xe*Wr + x*We); main PSUM carries 2^10 * (xr*Wr)
Z_DESCALE = 2.0**-17


def build(ts: int = TS) -> bass.Bass:
    nt = ts // P
    nc = bacc.Bacc("TRN2", target_bir_lowering=False)

    x_dram = nc.dram_tensor("x", [ts, D], F32, kind="ExternalInput")
    w_dram = nc.dram_tensor("w", [D, E], F32, kind="ExternalInput")
    b_dram = nc.dram_tensor("bias", [E], F32, kind="ExternalInput")
    ow_dram = nc.dram_tensor("out_w", [ts, TOP_K], F32, kind="ExternalOutput")
    oi_dram = nc.dram_tensor("out_i", [ts, TOP_K], I32, kind="ExternalOutput")

    with TileContext(nc) as tc:
        with (
            tc.tile_pool(name="consts", bufs=1) as cp,
            tc.tile_pool(name="natp", bufs=7) as natp,
            tc.tile_pool(name="xtp", bufs=3) as xtp,
            tc.tile_pool(name="x8p", bufs=3) as x8p,
            tc.tile_pool(name="wstg", bufs=4) as wstgp,
            tc.tile_pool(name="wep", bufs=2) as wep,
            tc.tile_pool(name="stg", bufs=2, space=bass.MemorySpace.PSUM) as stgp,
            tc.tile_pool(name="zp", bufs=2, space=bass.MemorySpace.PSUM) as zpp,
            tc.tile_pool(name="zcp", bufs=2, space=bass.MemorySpace.PSUM) as zcp,
            tc.tile_pool(name="sc", bufs=2) as scp,
            tc.tile_pool(name="rt", bufs=2) as rp,
            tc.tile_pool(name="outp", bufs=3) as op_,
        ):
            # ---- constants ----
            ident = cp.tile([P, P], F32)
            make_identity(nc, ident)

            bias_rep = cp.tile([P, E], F32)
            nc.gpsimd.dma_start(
                out=bias_rep,
                in_=bass.AP(tensor=b_dram, offset=0, ap=[[0, P], [1, E]]),
            )

            iota_i = cp.tile([P, E], I32)
            nc.gpsimd.iota(iota_i, pattern=[[1, E]], base=0, channel_multiplier=0)
            iota_f = cp.tile([P, E], F32)
            nc.vector.tensor_copy(iota_f, iota_i)

            # x eighth tiles, aligned 1:1 with transpose groups
            nat_tiles: dict[tuple, object] = {}

            def load_eighth(i, g):
                natq = natp.tile([P, TG * P], F32, tag="natq", name="natq")
                nat_tiles[(i, g)] = natq
                nc.sync.dma_start(
                    out=natq,
                    in_=x_dram[i * P : (i + 1) * P, g * TG * P : (g + 1) * TG * P],
                )

            # ---- resident weights: W_r (fp32r) and fp8 [W_r8 | W_e8] pairs ----
            wr_res = cp.tile([P, KC, E], F32R)
            w8_res = cp.tile([P, KC, 2, E], F8)
            w_re = w_dram.rearrange("(c p) e -> p c e", p=P)

            def load_w_group(wi):
                wfull = wstgp.tile([P, TG, E], F32, tag="wfull", name="wfull")
                nc.sync.dma_start(out=wfull, in_=w_re[:, wi : wi + TG, :])
                wr = wr_res[:, wi : wi + TG, :]
                nc.scalar.copy(wr, wfull)  # rounds fp32 -> fp32r
                we = wep.tile([P, TG, E], F32, tag="we", name="we")
                nc.vector.scalar_tensor_tensor(
                    we,
                    wfull,
                    1.0,
                    wr,
                    op0=mybir.AluOpType.mult,
                    op1=mybir.AluOpType.subtract,
                )
                nc.scalar.activation(
                    w8_res[:, wi : wi + TG, 0, :],
                    wfull,
                    mybir.ActivationFunctionType.Copy,
                    scale=SH_W8,
                )
                nc.scalar.activation(
                    w8_res[:, wi : wi + TG, 1, :],
                    we,
                    mybir.ActivationFunctionType.Copy,
                    scale=SH_WE8,
                )

            # Split W-group load: DMA + fp32r round early (cheap ACT),
            # the two fp8 converts (ACT-heavy) deferred into the step
            # stream so tile 0's xr_s ops aren't stuck behind ~40us of
            # W prep on the serial scalar queue.
            w_stage: dict[int, object] = {}

            def load_w_dma(wi):
                wfull = wstgp.tile([P, TG, E], F32, tag="wfull", name="wfull")
                w_stage[wi] = wfull
                nc.sync.dma_start(out=wfull, in_=w_re[:, wi : wi + TG, :])

            HG = TG // 2  # half-group of chunks for finer warmup interleave

            def prep_w_half(wi, h):
                wfull = w_stage[wi]
                lo, hi = wi + h * HG, wi + (h + 1) * HG
                wr = wr_res[:, lo:hi, :]
                # ACT: fp32r round + W_r8 convert
                nc.scalar.copy(wr, wfull[:, h * HG : (h + 1) * HG, :])
                nc.scalar.activation(
                    w8_res[:, lo:hi, 0, :],
                    wfull[:, h * HG : (h + 1) * HG, :],
                    mybir.ActivationFunctionType.Copy,
                    scale=SH_W8,
                )
                # DVE: residual + W_e8 convert
                we = wep.tile([P, HG, E], F32, tag="we", name="we")
                nc.vector.scalar_tensor_tensor(
                    we,
                    wfull[:, h * HG : (h + 1) * HG, :],
                    1.0,
                    wr,
                    op0=mybir.AluOpType.mult,
                    op1=mybir.AluOpType.subtract,
                )
                nc.vector.tensor_scalar(
                    w8_res[:, lo:hi, 1, :],
                    we,
                    SH_WE8,
                    None,
                    op0=mybir.AluOpType.mult,
                )
                if h == 1:
                    w_stage.pop(wi)

            def prep_w_group(wi):
                prep_w_half(wi, 0)
                prep_w_half(wi, 1)

            # interleave first x tile and W on the DMA queue: transposes need
            # x quarters promptly, the lagged matmuls need early W groups
            load_eighth(0, 0)
            load_eighth(0, 1)
            load_w_dma(0)
            load_eighth(0, 2)
            load_w_dma(8)
            load_eighth(0, 3)
            load_w_dma(16)
            load_eighth(0, 4)
            load_eighth(0, 5)
            load_eighth(0, 6)
            prep_w_group(0)

            xt_tiles: dict[tuple, object] = {}
            z_tiles: dict[int, object] = {}

            def transpose_group(i, g, mm=None):
                # interleave the 8 transposes with the lagged group's 16
                # matmuls on the PE queue: transpose ldweights hide under
                # matmul streams instead of bunching up
                stage = stgp.tile([P, TG * P], F32, tag="stage")
                natq = nat_tiles[(i, g)]
                if mm is not None:
                    mi, mg = mm
                    xrm, x8m = xt_tiles.pop(mm)
                    if mi not in z_tiles:
                        z_tiles[mi] = (
                            zpp.tile([P, 512], F32, tag="z", name="z"),
                            zcp.tile([P, 512], F32, tag="zc", name="zc"),
                        )
                    z, zc = z_tiles[mi]
                for j in range(TG):
                    nc.tensor.transpose(
                        stage[:, j * P : (j + 1) * P],
                        natq[:, j * P : (j + 1) * P],
                        ident,
                    )
                    if mm is not None:
                        c = mg * TG + j
                        nc.tensor.matmul(
                            z[:, 0:E],
                            xrm[:, j * P : (j + 1) * P],
                            wr_res[:, c, :],
                            start=(c == 0),
                            stop=(c == KC - 1),
                        )
                        nc.tensor.matmul(
                            zc[:, 0:E],
                            x8m[:, j, :, :],
                            w8_res[:, c, :, :],
                            start=(c == 0),
                            stop=(c == KC - 1),
                            perf_mode=mybir.MatmulPerfMode.DoubleRow,
                        )
                xr = xtp.tile([P, TG * P], F32R, tag="xr", name="xr")
                x8 = x8p.tile([P, TG, 2, P], F8, tag="x8", name="x8")
                xt_tiles[(i, g)] = (xr, x8)
                # xr_s = fp32r(x^T * 2^10)
                nc.scalar.activation(
                    xr, stage, mybir.ActivationFunctionType.Copy, scale=SCALE_X
                )
                st3 = stage.rearrange("p (g q) -> p g q", q=P)
                xr3 = xr.rearrange("p (g q) -> p g q", q=P)
                # xe8 = fp8((x^T * 2^10) - xr_s) = fp8(xe * 2^10)
                nc.vector.scalar_tensor_tensor(
                    x8[:, :, 0, :],
                    st3,
                    SCALE_X,
                    xr3,
                    op0=mybir.AluOpType.mult,
                    op1=mybir.AluOpType.subtract,
                )
                # x8 = fp8(xr_s * 2^-10) = fp8(x_r)
                nc.scalar.activation(
                    x8[:, :, 1, :],
                    xr3,
                    mybir.ActivationFunctionType.Copy,
                    scale=1.0 / SCALE_X,
                )
                nat_tiles.pop((i, g))

            def matmul_group(i, g):
                xr, x8 = xt_tiles.pop((i, g))
                if i not in z_tiles:
                    z_tiles[i] = (
                        zpp.tile([P, 512], F32, tag="z", name="z"),
                        zcp.tile([P, 512], F32, tag="zc", name="zc"),
                    )
                z, zc = z_tiles[i]
                for j in range(TG):
                    c = g * TG + j
                    nc.tensor.matmul(
                        z[:, 0:E],
                        xr[:, j * P : (j + 1) * P],
                        wr_res[:, c, :],
                        start=(c == 0),
                        stop=(c == KC - 1),
                    )
                    nc.tensor.matmul(
                        zc[:, 0:E],
                        x8[:, j, :, :],
                        w8_res[:, c, :, :],
                        start=(c == 0),
                        stop=(c == KC - 1),
                        perf_mode=mybir.MatmulPerfMode.DoubleRow,
                    )

            def routing(i):
                z, zc = z_tiles.pop(i)
                # z_total*2^17 = z*2^7 + zc  (z carries 2^10*x*Wr, zc carries 2^17*corr)
                zcs = scp.tile([P, E], F32, tag="zcs")
                nc.scalar.copy(zcs, zc[:, 0:E])
                zcomb = scp.tile([P, E], F32, tag="zcomb")
                nc.vector.scalar_tensor_tensor(
                    zcomb,
                    z[:, 0:E],
                    128.0,
                    zcs,
                    op0=mybir.AluOpType.mult,
                    op1=mybir.AluOpType.add,
                )
                scores = scp.tile([P, E], F32, tag="scores")
                nc.scalar.activation(
                    scores,
                    zcomb,
                    mybir.ActivationFunctionType.Sigmoid,
                    scale=Z_DESCALE,
                )

                biased = rp.tile([P, E], F32, tag="biased")
                nc.vector.tensor_add(biased, scores, bias_rep)

                gmax = rp.tile([P, N_GROUPS * 8], F32, tag="gmax")
                for g in range(N_GROUPS):
                    nc.vector.max(
                        gmax[:, g * 8 : (g + 1) * 8],
                        biased[:, g * EPG : (g + 1) * EPG],
                    )
                gm3 = gmax.rearrange("p (g k) -> p g k", k=8)
                gsc = rp.tile([P, N_GROUPS], F32, tag="gsc")
                gsc3 = gsc.rearrange("p (g k) -> p g k", k=1)
                nc.vector.tensor_add(gsc3, gm3[:, :, 0:1], gm3[:, :, 1:2])

                g8 = rp.tile([P, 8], F32, tag="g8")
                nc.vector.max(g8, gsc)
                maskg = rp.tile([P, N_GROUPS], F32, tag="maskg")
                nc.vector.tensor_scalar(
                    maskg,
                    gsc,
                    g8[:, TOPK_GROUPS - 1 : TOPK_GROUPS],
                    None,
                    op0=mybir.AluOpType.is_ge,
                )

                masked = rp.tile([P, E], F32, tag="masked")
                mg3 = maskg.rearrange("p (g k) -> p g k", k=1)
                nc.vector.tensor_tensor(
                    masked.rearrange("p (g e) -> p g e", g=N_GROUPS),
                    biased.rearrange("p (g e) -> p g e", g=N_GROUPS),
                    mg3.to_broadcast([P, N_GROUPS, EPG]),
                    op=mybir.AluOpType.mult,
                )

                top8 = rp.tile([P, 8], F32, tag="top8")
                nc.vector.max(top8, masked)
                idx = rp.tile([P, 8], U32, tag="idx")
                nc.vector.max_index(idx, top8, masked)
                idxf = rp.tile([P, 8], F32, tag="idxf")
                nc.vector.tensor_copy(idxf, idx)

                wg = rp.tile([P, 8], F32, tag="wg")
                scratch = rp.tile([P, E], F32, tag="scratch")
                for k in range(TOP_K):
                    nc.vector.scalar_tensor_tensor(
                        scratch,
                        iota_f,
                        idxf[:, k : k + 1],
                        scores,
                        op0=mybir.AluOpType.is_equal,
                        op1=mybir.AluOpType.mult,
                        accum_out=wg[:, k : k + 1],
                    )

                ssum = rp.tile([P, 1], F32, tag="ssum")
                nc.vector.tensor_reduce(
                    ssum, wg, axis=mybir.AxisListType.X, op=mybir.AluOpType.add
                )
                nc.vector.tensor_scalar_add(ssum, ssum, 1e-20)
                rinv = rp.tile([P, 1], F32, tag="rinv")
                nc.vector.reciprocal(rinv, ssum)
                nc.vector.tensor_scalar_mul(rinv, rinv, SCALE)

                wout = op_.tile([P, TOP_K], F32, tag="wout")
                nc.vector.tensor_tensor(
                    wout, wg, rinv.to_broadcast([P, TOP_K]), op=mybir.AluOpType.mult
                )
                iout = op_.tile([P, TOP_K], I32, tag="iout")
                nc.vector.tensor_copy(iout, idx)

                nc.scalar.dma_start(out=ow_dram[i * P : (i + 1) * P, :], in_=wout)
                nc.scalar.dma_start(out=oi_dram[i * P : (i + 1) * P, :], in_=iout)

            # flat (tile, group) step stream; matmuls lag transposes by MM_LAG
            steps = [(i, g) for i in range(nt) for g in range(NG)]
            # W-prep half-groups interleaved so each group is ready one step
            # before its lagged matmuls, without saturating ACT/DVE at warmup
            prep_sched = {
                0: [(1, 0)],
                1: [(1, 1), (2, 0)],
                2: [(2, 1), (3, 0)],
                3: [(3, 1)],
                4: [(4, 0), (4, 1)],
                5: [(5, 0), (5, 1)],
                6: [(6, 0), (6, 1)],
            }
            for s, (i, g) in enumerate(steps):
                if i + 1 < nt:
                    load_eighth(i + 1, g)
                # W groups trickle in: DMA 3 steps ahead of prep
                if s + 3 < NG:
                    load_w_dma((s + 3) * TG)
                for wg_, h_ in prep_sched.get(s, []):
                    prep_w_half(wg_ * TG, h_)
                if s >= MM_LAG:
                    mi, mg = steps[s - MM_LAG]
                    transpose_group(i, g, mm=(mi, mg))
                    if mg == NG - 1:
                        routing(mi)
                else:
                    transpose_group(i, g)
            for s in range(len(steps) - MM_LAG, len(steps)):
                mi, mg = steps[s]
                matmul_group(mi, mg)
                if mg == NG - 1:
                    routing(mi)

    nc.compile()
    return nc


def kernel(x_TD: np.ndarray, kernel_DE: np.ndarray, bias_E: np.ndarray):
    nc = build(TS)
    x_TD = np.ascontiguousarray(x_TD, dtype=np.float32)
    kernel_DE = np.ascontiguousarray(kernel_DE, dtype=np.float32)
    bias_E = np.ascontiguousarray(bias_E, dtype=np.float32)
    in_maps = [
        {
            "x": x_TD[c * TS : (c + 1) * TS],
            "w": kernel_DE,
            "bias": bias_E,
        }
        for c in range(N_CORES)
    ]
    res = run_bass_kernel_spmd(nc, in_maps, list(range(N_CORES)))
    w = np.concatenate([r["out_w"] for r in res.results], axis=0)
    i = np.concatenate([r["out_i"] for r in res.results], axis=0)
    return w.astype(np.float32), i.astype(np.int32)


# revision 14
# speedup vs baseline: 1.1970x; 1.1970x over previous
"""DeepSeekV3 router (moe_routing) Bass kernel for 8x TRN2 NeuronCores.

Data-parallel over tokens (T sharded 8 ways); kernel_DE/bias_E replicated.

z = x@W via one fp32r main pass plus one fp8 DoubleRow pass:
  main: xr_s * W_r           (fp32r, 12-bit significand, scaled 2^10)
  corr: xe8 * W_r8 + x8 * W_e8  (fp8e4m3 DoubleRow, both terms in one pass)
with xe = x - fp32r(x), We = W - fp32r(W). Residual error ~2^-17 relative,
at 256+128 matmul cycles per 128x128 chunk instead of fp32's 1024.
"""

import numpy as np

import concourse.bass as bass
import concourse.mybir as mybir
from concourse import bacc
from concourse.bass_utils import run_bass_kernel_spmd
from concourse.masks import make_identity
from concourse.tile import TileContext

F32 = mybir.dt.float32
F32R = mybir.dt.float32r
F8 = mybir.dt.float8e4
I32 = mybir.dt.int32
U32 = mybir.dt.uint32

T, D, E = 16384, 7168, 256
N_CORES = 8
TOP_K = 8
N_GROUPS = 8
TOPK_GROUPS = 4
EPG = E // N_GROUPS
SCALE = 2.5

P = 128
TS = T // N_CORES
KC = D // P                # 56 contraction chunks
TG = 8                     # chunks per group (PSUM stage + xt granularity)
NG = KC // TG              # 7 groups per tile
MM_LAG = 2                 # matmul groups lag transposes by this many steps

SCALE_X = 2.0**10          # x upscale inside xr_s (undone after matmul)
SH_W8 = 2.0**7             # W_r8 fp8 scale
SH_WE8 = 2.0**17           # W_e8 fp8 scale
# corr PSUM carries 2^17 * (xe*Wr + x*We); main PSUM carries 2^10 * (xr*Wr)
Z_DESCALE = 2.0**-17


def build(ts: int = TS) -> bass.Bass:
    nt = ts // P
    nc = bacc.Bacc("TRN2", target_bir_lowering=False)

    x_dram = nc.dram_tensor("x", [ts, D], F32, kind="ExternalInput")
    w_dram = nc.dram_tensor("w", [D, E], F32, kind="ExternalInput")
    b_dram = nc.dram_tensor("bias", [E], F32, kind="ExternalInput")
    ow_dram = nc.dram_tensor("out_w", [ts, TOP_K], F32, kind="ExternalOutput")
    oi_dram = nc.dram_tensor("out_i", [ts, TOP_K], I32, kind="ExternalOutput")

    with TileContext(nc) as tc:
        with (
            tc.tile_pool(name="consts", bufs=1) as cp,
            tc.tile_pool(name="natp", bufs=7) as natp,
            tc.tile_pool(name="xtp", bufs=3) as xtp,
            tc.tile_pool(name="x8p", bufs=3) as x8p,
            tc.tile_pool(name="wstg", bufs=4) as wstgp,
            tc.tile_pool(name="wep", bufs=2) as wep,
            tc.tile_pool(name="stg", bufs=2, space=bass.MemorySpace.PSUM) as stgp,
            tc.tile_pool(name="zp", bufs=2, space=bass.MemorySpace.PSUM) as zpp,
            tc.tile_pool(name="zcp", bufs=2, space=bass.MemorySpace.PSUM) as zcp,
            tc.tile_pool(name="sc", bufs=2) as scp,
            tc.tile_pool(name="rt", bufs=2) as rp,
            tc.tile_pool(name="outp", bufs=3) as op_,
        ):
            # ---- constants ----
            ident = cp.tile([P, P], F32)
            make_identity(nc, ident)

            bias_rep = cp.tile([P, E], F32)
            nc.gpsimd.dma_start(
                out=bias_rep,
                in_=bass.AP(tensor=b_dram, offset=0, ap=[[0, P], [1, E]]),
            )

            iota_i = cp.tile([P, E], I32)
            nc.gpsimd.iota(iota_i, pattern=[[1, E]], base=0, channel_multiplier=0)
            iota_f = cp.tile([P, E], F32)
            nc.vector.tensor_copy(iota_f, iota_i)

            # x eighth tiles, aligned 1:1 with transpose groups
            nat_tiles: dict[tuple, object] = {}

            def load_eighth(i, g):
                natq = natp.tile([P, TG * P], F32, tag="natq", name="natq")
                nat_tiles[(i, g)] = natq
                nc.sync.dma_start(
                    out=natq,
                    in_=x_dram[i * P : (i + 1) * P, g * TG * P : (g + 1) * TG * P],
                )

            # ---- resident weights: W_r (fp32r) and fp8 [W_r8 | W_e8] pairs ----
            wr_res = cp.tile([P, KC, E], F32R)
            w8_res = cp.tile([P, KC, 2, E], F8)
            w_re = w_dram.rearrange("(c p) e -> p c e", p=P)

            def load_w_group(wi):
                wfull = wstgp.tile([P, TG, E], F32, tag="wfull", name="wfull")
                nc.sync.dma_start(out=wfull, in_=w_re[:, wi : wi + TG, :])
                wr = wr_res[:, wi : wi + TG, :]
                nc.scalar.copy(wr, wfull)  # rounds fp32 -> fp32r
                we = wep.tile([P, TG, E], F32, tag="we", name="we")
                nc.vector.scalar_tensor_tensor(
                    we,
                    wfull,
                    1.0,
                    wr,
                    op0=mybir.AluOpType.mult,
                    op1=mybir.AluOpType.subtract,
                )
                nc.scalar.activation(
                    w8_res[:, wi : wi + TG, 0, :],
                    wfull,
                    mybir.ActivationFunctionType.Copy,
                    scale=SH_W8,
                )
                nc.scalar.activation(
                    w8_res[:, wi : wi + TG, 1, :],
                    we,
                    mybir.ActivationFunctionType.Copy,
                    scale=SH_WE8,
                )

            # Split W-group load: DMA + fp32r round early (cheap ACT),
            # the two fp8 converts (ACT-heavy) deferred into the step
            # stream so tile 0's xr_s ops aren't stuck behind ~40us of
            # W prep on the serial scalar queue.
            w_stage: dict[int, object] = {}

            def load_w_dma(wi):
                wfull = wstgp.tile([P, TG, E], F32, tag="wfull", name="wfull")
                w_stage[wi] = wfull
                nc.sync.dma_start(out=wfull, in_=w_re[:, wi : wi + TG, :])

            HG = TG // 2  # half-group of chunks for finer warmup interleave

            def prep_w_half(wi, h):
                wfull = w_stage[wi]
                lo, hi = wi + h * HG, wi + (h + 1) * HG
                wr = wr_res[:, lo:hi, :]
                # ACT: fp32r round + W_r8 convert
                nc.scalar.copy(wr, wfull[:, h * HG : (h + 1) * HG, :])
                nc.scalar.activation(
                    w8_res[:, lo:hi, 0, :],
                    wfull[:, h * HG : (h + 1) * HG, :],
                    mybir.ActivationFunctionType.Copy,
                    scale=SH_W8,
                )
                # DVE: residual + W_e8 convert
                we = wep.tile([P, HG, E], F32, tag="we", name="we")
                nc.vector.scalar_tensor_tensor(
                    we,
                    wfull[:, h * HG : (h + 1) * HG, :],
                    1.0,
                    wr,
                    op0=mybir.AluOpType.mult,
                    op1=mybir.AluOpType.subtract,
                )
                nc.vector.tensor_scalar(
                    w8_res[:, lo:hi, 1, :],
                    we,
                    SH_WE8,
                    None,
                    op0=mybir.AluOpType.mult,
                )
                if h == 1:
                    w_stage.pop(wi)

            def prep_w_group(wi):
                prep_w_half(wi, 0)
                prep_w_half(wi, 1)

            # interleave first x tile and W on the DMA queue: transposes need
            # x quarters promptly, the lagged matmuls need early W groups
            load_eighth(0, 0)
            load_eighth(0, 1)
            load_w_dma(0)
            load_eighth(0, 2)
            load_w_dma(8)
            load_eighth(0, 3)
            load_w_dma(16)
            load_eighth(0, 4)
            load_eighth(0, 5)
            load_eighth(0, 6)
            prep_w_group(0)

            xt_tiles: dict[tuple, object] = {}
            z_tiles: dict[int, object] = {}

            def transpose_group(i, g, mm=None):
                # interleave the 8 transposes with the lagged group's 16
                # matmuls on the PE queue: transpose ldweights hide under
                # matmul streams instead of bunching up
                stage = stgp.tile([P, TG * P], F32, tag="stage")
                natq = nat_tiles[(i, g)]
                for j in range(TG):
                    nc.tensor.transpose(
                        stage[:, j * P : (j + 1) * P],
                        natq[:, j * P : (j + 1) * P],
                        ident,
                    )
                xr = xtp.tile([P, TG * P], F32R, tag="xr", name="xr")
                x8 = x8p.tile([P, TG, 2, P], F8, tag="x8", name="x8")
                xt_tiles[(i, g)] = (xr, x8)
                # xr_s = fp32r(x^T * 2^10)
                nc.scalar.activation(
                    xr, stage, mybir.ActivationFunctionType.Copy, scale=SCALE_X
                )
                st3 = stage.rearrange("p (g q) -> p g q", q=P)
                xr3 = xr.rearrange("p (g q) -> p g q", q=P)
                # xe8 = fp8((x^T * 2^10) - xr_s) = fp8(xe * 2^10)
                nc.vector.scalar_tensor_tensor(
                    x8[:, :, 0, :],
                    st3,
                    SCALE_X,
                    xr3,
                    op0=mybir.AluOpType.mult,
                    op1=mybir.AluOpType.subtract,
                )
                # x8 = fp8(xr_s * 2^-10) = fp8(x_r)
                nc.scalar.activation(
                    x8[:, :, 1, :],
                    xr3,
                    mybir.ActivationFunctionType.Copy,
                    scale=1.0 / SCALE_X,
                )
                nat_tiles.pop((i, g))

            def matmul_group(i, g):
                xr, x8 = xt_tiles.pop((i, g))
                if i not in z_tiles:
                    z_tiles[i] = (
                        zpp.tile([P, 512], F32, tag="z", name="z"),
                        zcp.tile([P, 512], F32, tag="zc", name="zc"),
                    )
                z, zc = z_tiles[i]
                # batch same-mode matmuls: 8 fp32r mains back-to-back, then
                # 8 fp8-DR corrections (PE mode switches are expensive)
                for j in range(TG):
                    c = g * TG + j
                    nc.tensor.matmul(
                        z[:, 0:E],
                        xr[:, j * P : (j + 1) * P],
                        wr_res[:, c, :],
                        start=(c == 0),
                        stop=(c == KC - 1),
                    )
                for j in range(TG):
                    c = g * TG + j
                    nc.tensor.matmul(
                        zc[:, 0:E],
                        x8[:, j, :, :],
                        w8_res[:, c, :, :],
                        start=(c == 0),
                        stop=(c == KC - 1),
                        perf_mode=mybir.MatmulPerfMode.DoubleRow,
                    )

            def routing(i):
                z, zc = z_tiles.pop(i)
                # z_total*2^17 = z*2^7 + zc  (z carries 2^10*x*Wr, zc carries 2^17*corr)
                zcs = scp.tile([P, E], F32, tag="zcs")
                nc.scalar.copy(zcs, zc[:, 0:E])
                zcomb = scp.tile([P, E], F32, tag="zcomb")
                nc.vector.scalar_tensor_tensor(
                    zcomb,
                    z[:, 0:E],
                    128.0,
                    zcs,
                    op0=mybir.AluOpType.mult,
                    op1=mybir.AluOpType.add,
                )
                scores = scp.tile([P, E], F32, tag="scores")
                nc.scalar.activation(
                    scores,
                    zcomb,
                    mybir.ActivationFunctionType.Sigmoid,
                    scale=Z_DESCALE,
                )

                biased = rp.tile([P, E], F32, tag="biased")
                nc.vector.tensor_add(biased, scores, bias_rep)

                gmax = rp.tile([P, N_GROUPS * 8], F32, tag="gmax")
                for g in range(N_GROUPS):
                    nc.vector.max(
                        gmax[:, g * 8 : (g + 1) * 8],
                        biased[:, g * EPG : (g + 1) * EPG],
                    )
                gm3 = gmax.rearrange("p (g k) -> p g k", k=8)
                gsc = rp.tile([P, N_GROUPS], F32, tag="gsc")
                gsc3 = gsc.rearrange("p (g k) -> p g k", k=1)
                nc.vector.tensor_add(gsc3, gm3[:, :, 0:1], gm3[:, :, 1:2])

                g8 = rp.tile([P, 8], F32, tag="g8")
                nc.vector.max(g8, gsc)
                maskg = rp.tile([P, N_GROUPS], F32, tag="maskg")
                nc.vector.tensor_scalar(
                    maskg,
                    gsc,
                    g8[:, TOPK_GROUPS - 1 : TOPK_GROUPS],
                    None,
                    op0=mybir.AluOpType.is_ge,
                )

                masked = rp.tile([P, E], F32, tag="masked")
                mg3 = maskg.rearrange("p (g k) -> p g k", k=1)
                nc.vector.tensor_tensor(
                    masked.rearrange("p (g e) -> p g e", g=N_GROUPS),
                    biased.rearrange("p (g e) -> p g e", g=N_GROUPS),
                    mg3.to_broadcast([P, N_GROUPS, EPG]),
                    op=mybir.AluOpType.mult,
                )

                top8 = rp.tile([P, 8], F32, tag="top8")
                nc.vector.max(top8, masked)
                idx = rp.tile([P, 8], U32, tag="idx")
                nc.vector.max_index(idx, top8, masked)
                idxf = rp.tile([P, 8], F32, tag="idxf")
                nc.vector.tensor_copy(idxf, idx)

                wg = rp.tile([P, 8], F32, tag="wg")
                scratch = rp.tile([P, E], F32, tag="scratch")
                for k in range(TOP_K):
                    nc.vector.scalar_tensor_tensor(
                        scratch,
                        iota_f,
                        idxf[:, k : k + 1],
                        scores,
                        op0=mybir.AluOpType.is_equal,
                        op1=mybir.AluOpType.mult,
                        accum_out=wg[:, k : k + 1],
                    )

                ssum = rp.tile([P, 1], F32, tag="ssum")
                nc.vector.tensor_reduce(
                    ssum, wg, axis=mybir.AxisListType.X, op=mybir.AluOpType.add
                )
                nc.vector.tensor_scalar_add(ssum, ssum, 1e-20)
                rinv = rp.tile([P, 1], F32, tag="rinv")
                nc.vector.reciprocal(rinv, ssum)
                nc.vector.tensor_scalar_mul(rinv, rinv, SCALE)

                wout = op_.tile([P, TOP_K], F32, tag="wout")
                nc.vector.tensor_tensor(
                    wout, wg, rinv.to_broadcast([P, TOP_K]), op=mybir.AluOpType.mult
                )
                iout = op_.tile([P, TOP_K], I32, tag="iout")
                nc.vector.tensor_copy(iout, idx)

                nc.scalar.dma_start(out=ow_dram[i * P : (i + 1) * P, :], in_=wout)
                nc.scalar.dma_start(out=oi_dram[i * P : (i + 1) * P, :], in_=iout)

            # flat (tile, group) step stream; matmuls lag transposes by MM_LAG
            steps = [(i, g) for i in range(nt) for g in range(NG)]
            # W-prep half-groups interleaved so each group is ready one step
            # before its lagged matmuls, without saturating ACT/DVE at warmup
            prep_sched = {
                0: [(1, 0)],
                1: [(1, 1), (2, 0)],
                2: [(2, 1), (3, 0)],
                3: [(3, 1)],
                4: [(4, 0), (4, 1)],
                5: [(5, 0), (5, 1)],
                6: [(6, 0), (6, 1)],
            }
            for s, (i, g) in enumerate(steps):
                if i + 1 < nt:
                    load_eighth(i + 1, g)
                # W groups trickle in: DMA 3 steps ahead of prep
                if s + 3 < NG:
                    load_w_dma((s + 3) * TG)
                for wg_, h_ in prep_sched.get(s, []):
                    prep_w_half(wg_ * TG, h_)
                transpose_group(i, g)
                if s >= MM_LAG:
                    mi, mg = steps[s - MM_LAG]
                    matmul_group(mi, mg)
                    if mg == NG - 1:
                        routing(mi)
            for s in range(len(steps) - MM_LAG, len(steps)):
                mi, mg = steps[s]
                matmul_group(mi, mg)
                if mg == NG - 1:
                    routing(mi)

    nc.compile()
    return nc


def kernel(x_TD: np.ndarray, kernel_DE: np.ndarray, bias_E: np.ndarray):
    nc = build(TS)
    x_TD = np.ascontiguousarray(x_TD, dtype=np.float32)
    kernel_DE = np.ascontiguousarray(kernel_DE, dtype=np.float32)
    bias_E = np.ascontiguousarray(bias_E, dtype=np.float32)
    in_maps = [
        {
            "x": x_TD[c * TS : (c + 1) * TS],
            "w": kernel_DE,
            "bias": bias_E,
        }
        for c in range(N_CORES)
    ]
    res = run_bass_kernel_spmd(nc, in_maps, list(range(N_CORES)))
    w = np.concatenate([r["out_w"] for r in res.results], axis=0)
    i = np.concatenate([r["out_i"] for r in res.results], axis=0)
    return w.astype(np.float32), i.astype(np.int32)


# revision 15
# speedup vs baseline: 1.2017x; 1.0039x over previous
"""DeepSeekV3 router (moe_routing) Bass kernel for 8x TRN2 NeuronCores.

Data-parallel over tokens (T sharded 8 ways); kernel_DE/bias_E replicated.

z = x@W via one fp32r main pass plus one fp8 DoubleRow pass:
  main: xr_s * W_r           (fp32r, 12-bit significand, scaled 2^10)
  corr: xe8 * W_r8 + x8 * W_e8  (fp8e4m3 DoubleRow, both terms in one pass)
with xe = x - fp32r(x), We = W - fp32r(W). Residual error ~2^-17 relative,
at 256+128 matmul cycles per 128x128 chunk instead of fp32's 1024.
"""

import numpy as np

import concourse.bass as bass
import concourse.mybir as mybir
from concourse import bacc
from concourse.bass_utils import run_bass_kernel_spmd
from concourse.masks import make_identity
from concourse.tile import TileContext

F32 = mybir.dt.float32
F32R = mybir.dt.float32r
F8 = mybir.dt.float8e4
I32 = mybir.dt.int32
U32 = mybir.dt.uint32

T, D, E = 16384, 7168, 256
N_CORES = 8
TOP_K = 8
N_GROUPS = 8
TOPK_GROUPS = 4
EPG = E // N_GROUPS
SCALE = 2.5

P = 128
TS = T // N_CORES
KC = D // P                # 56 contraction chunks
TG = 8                     # chunks per group (PSUM stage + xt granularity)
NG = KC // TG              # 7 groups per tile
MM_LAG = 2                 # matmul groups lag transposes by this many steps

SCALE_X = 2.0**10          # x upscale inside xr_s (undone after matmul)
SH_W8 = 2.0**7             # W_r8 fp8 scale
SH_WE8 = 2.0**17           # W_e8 fp8 scale
# corr PSUM carries 2^17 * (xe*Wr + x*We); main PSUM carries 2^10 * (xr*Wr)
Z_DESCALE = 2.0**-17


def build(ts: int = TS) -> bass.Bass:
    nt = ts // P
    nc = bacc.Bacc("TRN2", target_bir_lowering=False)

    x_dram = nc.dram_tensor("x", [ts, D], F32, kind="ExternalInput")
    w_dram = nc.dram_tensor("w", [D, E], F32, kind="ExternalInput")
    b_dram = nc.dram_tensor("bias", [E], F32, kind="ExternalInput")
    ow_dram = nc.dram_tensor("out_w", [ts, TOP_K], F32, kind="ExternalOutput")
    oi_dram = nc.dram_tensor("out_i", [ts, TOP_K], I32, kind="ExternalOutput")

    with TileContext(nc) as tc:
        with (
            tc.tile_pool(name="consts", bufs=1) as cp,
            tc.tile_pool(name="natp", bufs=7) as natp,
            tc.tile_pool(name="xtp", bufs=3) as xtp,
            tc.tile_pool(name="x8p", bufs=3) as x8p,
            tc.tile_pool(name="wstg", bufs=4) as wstgp,
            tc.tile_pool(name="wep", bufs=2) as wep,
            tc.tile_pool(name="stg", bufs=2, space=bass.MemorySpace.PSUM) as stgp,
            tc.tile_pool(name="zp", bufs=2, space=bass.MemorySpace.PSUM) as zpp,
            tc.tile_pool(name="zcp", bufs=2, space=bass.MemorySpace.PSUM) as zcp,
            tc.tile_pool(name="sc", bufs=2) as scp,
            tc.tile_pool(name="rt", bufs=2) as rp,
            tc.tile_pool(name="outp", bufs=3) as op_,
        ):
            # ---- constants ----
            ident = cp.tile([P, P], F32)
            make_identity(nc, ident)

            bias_rep = cp.tile([P, E], F32)
            nc.gpsimd.dma_start(
                out=bias_rep,
                in_=bass.AP(tensor=b_dram, offset=0, ap=[[0, P], [1, E]]),
            )

            iota_i = cp.tile([P, E], I32)
            nc.gpsimd.iota(iota_i, pattern=[[1, E]], base=0, channel_multiplier=0)
            iota_f = cp.tile([P, E], F32)
            nc.vector.tensor_copy(iota_f, iota_i)

            # x eighth tiles, aligned 1:1 with transpose groups
            nat_tiles: dict[tuple, object] = {}

            def load_eighth(i, g):
                natq = natp.tile([P, TG * P], F32, tag="natq", name="natq")
                nat_tiles[(i, g)] = natq
                nc.sync.dma_start(
                    out=natq,
                    in_=x_dram[i * P : (i + 1) * P, g * TG * P : (g + 1) * TG * P],
                )

            # ---- resident weights: W_r (fp32r) and fp8 [W_r8 | W_e8] pairs ----
            wr_res = cp.tile([P, KC, E], F32R)
            w8_res = cp.tile([P, KC, 2, E], F8)
            w_re = w_dram.rearrange("(c p) e -> p c e", p=P)

            def load_w_group(wi):
                wfull = wstgp.tile([P, TG, E], F32, tag="wfull", name="wfull")
                nc.sync.dma_start(out=wfull, in_=w_re[:, wi : wi + TG, :])
                wr = wr_res[:, wi : wi + TG, :]
                nc.scalar.copy(wr, wfull)  # rounds fp32 -> fp32r
                we = wep.tile([P, TG, E], F32, tag="we", name="we")
                nc.vector.scalar_tensor_tensor(
                    we,
                    wfull,
                    1.0,
                    wr,
                    op0=mybir.AluOpType.mult,
                    op1=mybir.AluOpType.subtract,
                )
                nc.scalar.activation(
                    w8_res[:, wi : wi + TG, 0, :],
                    wfull,
                    mybir.ActivationFunctionType.Copy,
                    scale=SH_W8,
                )
                nc.scalar.activation(
                    w8_res[:, wi : wi + TG, 1, :],
                    we,
                    mybir.ActivationFunctionType.Copy,
                    scale=SH_WE8,
                )

            # Split W-group load: DMA + fp32r round early (cheap ACT),
            # the two fp8 converts (ACT-heavy) deferred into the step
            # stream so tile 0's xr_s ops aren't stuck behind ~40us of
            # W prep on the serial scalar queue.
            w_stage: dict[int, object] = {}

            def load_w_dma(wi):
                wfull = wstgp.tile([P, TG, E], F32, tag="wfull", name="wfull")
                w_stage[wi] = wfull
                nc.sync.dma_start(out=wfull, in_=w_re[:, wi : wi + TG, :])

            HG = TG // 2  # half-group of chunks for finer warmup interleave

            def prep_w_half(wi, h):
                wfull = w_stage[wi]
                lo, hi = wi + h * HG, wi + (h + 1) * HG
                wr = wr_res[:, lo:hi, :]
                # ACT: fp32r round + W_r8 convert
                nc.scalar.copy(wr, wfull[:, h * HG : (h + 1) * HG, :])
                nc.scalar.activation(
                    w8_res[:, lo:hi, 0, :],
                    wfull[:, h * HG : (h + 1) * HG, :],
                    mybir.ActivationFunctionType.Copy,
                    scale=SH_W8,
                )
                # DVE: residual + W_e8 convert
                we = wep.tile([P, HG, E], F32, tag="we", name="we")
                nc.vector.scalar_tensor_tensor(
                    we,
                    wfull[:, h * HG : (h + 1) * HG, :],
                    1.0,
                    wr,
                    op0=mybir.AluOpType.mult,
                    op1=mybir.AluOpType.subtract,
                )
                nc.vector.tensor_scalar(
                    w8_res[:, lo:hi, 1, :],
                    we,
                    SH_WE8,
                    None,
                    op0=mybir.AluOpType.mult,
                )
                if h == 1:
                    w_stage.pop(wi)

            def prep_w_group(wi):
                prep_w_half(wi, 0)
                prep_w_half(wi, 1)

            # interleave first x tile and W on the DMA queue: transposes need
            # x quarters promptly, the lagged matmuls need early W groups
            load_eighth(0, 0)
            load_eighth(0, 1)
            load_w_dma(0)
            load_eighth(0, 2)
            load_w_dma(8)
            load_eighth(0, 3)
            load_w_dma(16)
            load_eighth(0, 4)
            load_eighth(0, 5)
            load_eighth(0, 6)
            prep_w_group(0)

            xt_tiles: dict[tuple, object] = {}
            z_tiles: dict[int, object] = {}

            def transpose_group(i, g, mm=None):
                # interleave the 8 transposes with the lagged group's 16
                # matmuls on the PE queue: transpose ldweights hide under
                # matmul streams instead of bunching up
                stage = stgp.tile([P, TG * P], F32, tag="stage")
                natq = nat_tiles[(i, g)]
                for j in range(TG):
                    nc.tensor.transpose(
                        stage[:, j * P : (j + 1) * P],
                        natq[:, j * P : (j + 1) * P],
                        ident,
                    )
                xr = xtp.tile([P, TG * P], F32R, tag="xr", name="xr")
                x8 = x8p.tile([P, TG, 2, P], F8, tag="x8", name="x8")
                xt_tiles[(i, g)] = (xr, x8)
                # xr_s = fp32r(x^T * 2^10)
                nc.scalar.activation(
                    xr, stage, mybir.ActivationFunctionType.Copy, scale=SCALE_X
                )
                st3 = stage.rearrange("p (g q) -> p g q", q=P)
                xr3 = xr.rearrange("p (g q) -> p g q", q=P)
                # xe8 = fp8((x^T * 2^10) - xr_s) = fp8(xe * 2^10)
                nc.vector.scalar_tensor_tensor(
                    x8[:, :, 0, :],
                    st3,
                    SCALE_X,
                    xr3,
                    op0=mybir.AluOpType.mult,
                    op1=mybir.AluOpType.subtract,
                )
                # x8 = fp8(xr_s * 2^-10) = fp8(x_r)
                nc.scalar.activation(
                    x8[:, :, 1, :],
                    xr3,
                    mybir.ActivationFunctionType.Copy,
                    scale=1.0 / SCALE_X,
                )
                nat_tiles.pop((i, g))

            def matmul_group(i, g):
                xr, x8 = xt_tiles.pop((i, g))
                if i not in z_tiles:
                    z_tiles[i] = (
                        zpp.tile([P, 512], F32, tag="z", name="z"),
                        zcp.tile([P, 512], F32, tag="zc", name="zc"),
                    )
                z, zc = z_tiles[i]
                # batch same-mode matmuls: 8 fp32r mains back-to-back, then
                # 8 fp8-DR corrections (PE mode switches are expensive)
                for j in range(TG):
                    c = g * TG + j
                    nc.tensor.matmul(
                        z[:, 0:E],
                        xr[:, j * P : (j + 1) * P],
                        wr_res[:, c, :],
                        start=(c == 0),
                        stop=(c == KC - 1),
                    )
                for j in range(TG):
                    c = g * TG + j
                    nc.tensor.matmul(
                        zc[:, 0:E],
                        x8[:, j, :, :],
                        w8_res[:, c, :, :],
                        start=(c == 0),
                        stop=(c == KC - 1),
                        perf_mode=mybir.MatmulPerfMode.DoubleRow,
                    )

            def routing(i):
                z, zc = z_tiles.pop(i)
                # z_total*2^17 = z*2^7 + zc  (z carries 2^10*x*Wr, zc carries 2^17*corr)
                zcs = scp.tile([P, E], F32, tag="zcs")
                nc.scalar.copy(zcs, zc[:, 0:E])
                zcomb = scp.tile([P, E], F32, tag="zcomb")
                nc.vector.scalar_tensor_tensor(
                    zcomb,
                    z[:, 0:E],
                    128.0,
                    zcs,
                    op0=mybir.AluOpType.mult,
                    op1=mybir.AluOpType.add,
                )
                scores = scp.tile([P, E], F32, tag="scores")
                nc.scalar.activation(
                    scores,
                    zcomb,
                    mybir.ActivationFunctionType.Sigmoid,
                    scale=Z_DESCALE,
                )

                biased = rp.tile([P, E], F32, tag="biased")
                nc.vector.tensor_add(biased, scores, bias_rep)

                gmax = rp.tile([P, N_GROUPS * 8], F32, tag="gmax")
                for g in range(N_GROUPS):
                    nc.vector.max(
                        gmax[:, g * 8 : (g + 1) * 8],
                        biased[:, g * EPG : (g + 1) * EPG],
                    )
                gm3 = gmax.rearrange("p (g k) -> p g k", k=8)
                gsc = rp.tile([P, N_GROUPS], F32, tag="gsc")
                gsc3 = gsc.rearrange("p (g k) -> p g k", k=1)
                nc.vector.tensor_add(gsc3, gm3[:, :, 0:1], gm3[:, :, 1:2])

                g8 = rp.tile([P, 8], F32, tag="g8")
                nc.vector.max(g8, gsc)
                maskg = rp.tile([P, N_GROUPS], F32, tag="maskg")
                nc.vector.tensor_scalar(
                    maskg,
                    gsc,
                    g8[:, TOPK_GROUPS - 1 : TOPK_GROUPS],
                    None,
                    op0=mybir.AluOpType.is_ge,
                )

                masked = rp.tile([P, E], F32, tag="masked")
                mg3 = maskg.rearrange("p (g k) -> p g k", k=1)
                nc.vector.tensor_tensor(
                    masked.rearrange("p (g e) -> p g e", g=N_GROUPS),
                    biased.rearrange("p (g e) -> p g e", g=N_GROUPS),
                    mg3.to_broadcast([P, N_GROUPS, EPG]),
                    op=mybir.AluOpType.mult,
                )

                top8 = rp.tile([P, 8], F32, tag="top8")
                nc.vector.max(top8, masked)
                idx = rp.tile([P, 8], U32, tag="idx")
                nc.vector.max_index(idx, top8, masked)
                idxf = rp.tile([P, 8], F32, tag="idxf")
                nc.vector.tensor_copy(idxf, idx)

                wg = rp.tile([P, 8], F32, tag="wg")
                scratch = rp.tile([P, E], F32, tag="scratch")
                for k in range(TOP_K):
                    nc.vector.scalar_tensor_tensor(
                        scratch,
                        iota_f,
                        idxf[:, k : k + 1],
                        scores,
                        op0=mybir.AluOpType.is_equal,
                        op1=mybir.AluOpType.mult,
                        accum_out=wg[:, k : k + 1],
                    )

                ssum = rp.tile([P, 1], F32, tag="ssum")
                nc.vector.tensor_reduce(
                    ssum, wg, axis=mybir.AxisListType.X, op=mybir.AluOpType.add
                )
                nc.vector.tensor_scalar_add(ssum, ssum, 1e-20)
                rinv = rp.tile([P, 1], F32, tag="rinv")
                nc.vector.reciprocal(rinv, ssum)
                nc.vector.tensor_scalar_mul(rinv, rinv, SCALE)

                wout = op_.tile([P, TOP_K], F32, tag="wout")
                nc.vector.tensor_tensor(
                    wout, wg, rinv.to_broadcast([P, TOP_K]), op=mybir.AluOpType.mult
                )
                iout = op_.tile([P, TOP_K], I32, tag="iout")
                nc.vector.tensor_copy(iout, idx)

                nc.sync.dma_start(out=ow_dram[i * P : (i + 1) * P, :], in_=wout)
                nc.sync.dma_start(out=oi_dram[i * P : (i + 1) * P, :], in_=iout)

            # flat (tile, group) step stream; matmuls lag transposes by MM_LAG
            steps = [(i, g) for i in range(nt) for g in range(NG)]
            # W-prep half-groups interleaved so each group is ready one step
            # before its lagged matmuls, without saturating ACT/DVE at warmup
            prep_sched = {
                0: [(1, 0)],
                1: [(1, 1), (2, 0)],
                2: [(2, 1), (3, 0)],
                3: [(3, 1)],
                4: [(4, 0), (4, 1)],
                5: [(5, 0), (5, 1)],
                6: [(6, 0), (6, 1)],
            }
            for s, (i, g) in enumerate(steps):
                if i + 1 < nt:
                    load_eighth(i + 1, g)
                # W groups trickle in: DMA 3 steps ahead of prep
                if s + 3 < NG:
                    load_w_dma((s + 3) * TG)
                for wg_, h_ in prep_sched.get(s, []):
                    prep_w_half(wg_ * TG, h_)
                transpose_group(i, g)
                if s >= MM_LAG:
                    mi, mg = steps[s - MM_LAG]
                    matmul_group(mi, mg)
                    if mg == NG - 1:
                        routing(mi)
            for s in range(len(steps) - MM_LAG, len(steps)):
                mi, mg = steps[s]
                matmul_group(mi, mg)
                if mg == NG - 1:
                    routing(mi)

    nc.compile()
    return nc


def kernel(x_TD: np.ndarray, kernel_DE: np.ndarray, bias_E: np.ndarray):
    nc = build(TS)
    x_TD = np.ascontiguousarray(x_TD, dtype=np.float32)
    kernel_DE = np.ascontiguousarray(kernel_DE, dtype=np.float32)
    bias_E = np.ascontiguousarray(bias_E, dtype=np.float32)
    in_maps = [
        {
            "x": x_TD[c * TS : (c + 1) * TS],
            "w": kernel_DE,
            "bias": bias_E,
        }
        for c in range(N_CORES)
    ]
    res = run_bass_kernel_spmd(nc, in_maps, list(range(N_CORES)))
    w = np.concatenate([r["out_w"] for r in res.results], axis=0)
    i = np.concatenate([r["out_i"] for r in res.results], axis=0)
    return w.astype(np.float32), i.astype(np.int32)
